# revision 10
# baseline (speedup 1.0000x reference)
"""HGAT layer Trainium2 Bass kernel (fp8 DoubleRow + dual-path masked exp).

Math (per batch element b, per group pair):
  q,k,v = relu(x @ w + b) for each group
  4 masked attentions (00, 11, 01, 10), each NH=4 heads of H=32
  inner/inter = relu(attn @ wo + bo); out_g = concat(inner_g, inter_g) @ wf_g + bf_g

Device-side design (per core, 4 batch elements, data-parallel over B=32):
  - Transposed orientation (features on SBUF partitions). QKV projections fp16.
  - q/k fp8e4m3 in DoubleRow layout [128, 2, 512]: head h on partitions
    32h..32h+16, feature f at (partition 32h + f%16, t = f//16). scores^T per
    (chunk, head) in ONE DoubleRow matmul (256 cyc vs 512 fp16).
  - Two masked-exp paths, assignable per (b, pair) via ACT_SET:
    ACT path: mask folded into PSUM via DoubleRow [96I|96I] x broadcast-mask
      matmuls (256 cyc per half-bank), then pt = Exp(sc/sqrt(dk) - 192/sqrt(dk))
      on ACT writing fp8e4m3 directly (masked -> exp(-30) -> 0).
    DVE path (Schraudolph): pt_i8 = int8_sat(sc*A8 + (355.65*m - 300)) in one
      scalar_tensor_tensor; int8 bits reinterpreted as fp8e4m3 approximate
      exp(sc/sqrt(dk))*m (masked entries saturate to -128 = -0.0).
  - v fp8e4m3 zero-padded pair layout [128, 2(c2), 2(t), 2(pair), 96]:
    pair block = [f_even(32) | zeros(32) | f_odd(32)]. av/den are column-0
    DoubleRow matmuls (DR forbids nonzero dst partition): even head -> rows
    0:32, odd head (leading-zero lhsT) -> rows 0:64; denominators via a
    [zeros96|ones32] constant lhsT the same way, broadcast to 32 rows.
    av/den PSUM [64, 2(bank=head-pair), 512].
  - recip + normalize on DVE over [64, 2, 512]; wo via 2 accumulating matmuls
    (one per head-pair bank); wf unchanged. Output fp32, host transposes.
"""

import sys

sys.path.insert(0, "/opt/trn_rl_repo")

import numpy as np

import concourse.bacc as bacc
import concourse.tile as tile
from concourse import mybir

B, N, NH, H = 32, 512, 4, 32
IN_DIM, OUT_DIM = 128, 128
NCORES = 8
BS = B // NCORES  # batch elements per core
SQRT_DK = float(np.sqrt(H))
BIGC = 192.0  # ACT-path mask offset
A8 = 8.0 / (np.log(2.0) * SQRT_DK)  # Schraudolph slope for fp8e4m3
B8 = 7 * 8 - 0.35  # Schraudolph intercept (bias 7 << 3 mantissa bits, -C tweak)
MOFF = 300.0  # masked-out offset -> saturates int8 to -128 = -0.0
F32 = mybir.dt.float32
F16 = mybir.dt.float16
F8 = mybir.dt.float8e4
I8 = mybir.dt.int8
ADD = mybir.AluOpType.add
MAX = mybir.AluOpType.max
MULT = mybir.AluOpType.mult
EXP = mybir.ActivationFunctionType.Exp
DR = mybir.MatmulPerfMode.DoubleRow

# pair p -> (q group, k/v group); mask m{qg}{kg}; wo{qg}{kg}
PAIRS = [(0, 0), (1, 1), (0, 1), (1, 0)]
# pair -> (out group, concat row offset): inner pairs at rows 0:32, inter at 32:64
PAIR_DEST = [(0, 0), (1, 0), (0, 32), (1, 32)]
# exp path per (b, pair) flat index b*4+p: True -> ACT path, False -> DVE STT
ACT_SET = [True] * 16


def _emit_qkv(nc, pools, W, b, g, qt, kt, vt):
    """QKV for (b, g): qt/kt [128,2,512] fp8 DR layout, vt zero-padded fp8."""
    xt_t = pools["xt"].tile([128, N], F16, tag="xt", name="xt")
    nc.sync.dma_start(out=xt_t[:], in_=W["xt_ap"][b * 2 + g])

    for qk, (dst, bias) in enumerate(((qt, "bq"), (kt, "bk"))):
        for t in range(2):
            pp = pools["sc"].tile([128, N], F32, tag="sc", name="sc")
            nc.tensor.matmul(
                pp[:], W["wqk"][g][qk][t][:], xt_t[:], start=True, stop=True
            )
            nc.vector.tensor_scalar(
                dst[:, t, :], pp[:], W[bias][g][t][:], 0.0, op0=ADD, op1=MAX
            )

    # vp [128, 4(c2,t), 2(pair), 2(feat-parity), 32]; chunk c -> block 2*(c//2)+c%2 = c
    vp = pools["sc"].tile([128, 4, 2, 2, 32], F32, tag="sc", name="sc")
    # full-bank bias write opens the accumulation group (orders all chains)
    nc.tensor.matmul(
        vp[:, :, :, :, :], W["onesrow"][:], W["bvr4"][g][:], start=True, stop=False
    )
    for c in range(4):
        nc.tensor.matmul(
            vp[:, c, :, :, :],
            xt_t[:, 128 * c : 128 * (c + 1)],
            W["wv"][g][:],
            start=False,
            stop=False,
        )
    # full-bank +0 accumulate closes the group (runs after all chains)
    nc.tensor.matmul(vp[:, :, :, :, :], W["zrow16"][:], xt_t[0:1, :], start=False, stop=True)
    # strided relu epilogues: even feats -> pair cols 0:32, odd -> 64:96
    nc.vector.tensor_scalar_max(vt[:, :, :, :, 0:32], vp[:, :, :, 0, :], 0.0)
    nc.vector.tensor_scalar_max(vt[:, :, :, :, 64:96], vp[:, :, :, 1, :], 0.0)


def _emit_attn_b(nc, pools, W, b, qt, kt, vt, cc):
    """Emit the 4 attention pairs + wo for batch element b."""
    for p, (qg, kg) in enumerate(PAIRS):
        use_act = ACT_SET[b * 4 + p]
        pt = pools["pt"].tile([128, 2, 4, 2, 512], F8, tag="pt", name="pt")
        if use_act:
            mt_t = pools["mt"].tile([128, 4, N], F8, tag="mt", name="mt")
            nc.sync.dma_start(
                out=mt_t[:, :, :], in_=W["mt8_ap"][W["mt8_idx"][b * 4 + p]]
            )
        else:
            mt_t = pools["mt16"].tile([128, 4, 1024], F16, tag="mt16", name="mt16")
            nc.sync.dma_start(
                out=mt_t[:, :, :], in_=W["mt16_ap"][W["mt16_idx"][b * 4 + p]]
            )
        for c in range(4):
            c2, t = c // 2, c % 2
            for hh in range(2):
                sc = pools["sc"].tile([128, 1024], F32, tag="sc", name="sc")
                for j in range(2):
                    h = 2 * hh + j
                    nc.tensor.matmul(
                        sc[:, 512 * j : 512 * (j + 1)],
                        kt[kg][32 * h : 32 * h + 16, :, 128 * c : 128 * (c + 1)],
                        qt[qg][32 * h : 32 * h + 16, :, :],
                        start=True,
                        stop=not use_act,
                        tile_position=(32 * h, 0),
                        perf_mode=DR,
                        skip_group_check=True,
                    )
                if use_act:
                    # DR mask inject per half: sc[:, 512j:] += 192 * mT_c
                    mb = mt_t[:, c : c + 1, :].broadcast_to((128, 2, N))
                    for j in range(2):
                        nc.tensor.matmul(
                            sc[:, 512 * j : 512 * (j + 1)],
                            W["i96"][:, :, :],
                            mb,
                            start=False,
                            stop=True,
                            tile_position=(0, 0),
                            perf_mode=DR,
                            skip_group_check=True,
                        )
                    nc.scalar.activation(
                        pt[:, c2, 2 * hh : 2 * hh + 2, t, :],
                        sc[:],
                        EXP,
                        scale=1.0 / SQRT_DK,
                        bias=W["ebias"][:],
                    )
                else:
                    # Schraudolph: pt_i8 = int8_sat(sc*A8 + (355.65m - 300))
                    nc.vector.scalar_tensor_tensor(
                        pt[:, c2, 2 * hh : 2 * hh + 2, t, :].bitcast(I8),
                        sc[:],
                        float(A8),
                        mt_t[:, c, :],
                        op0=MULT,
                        op1=ADD,
                    )
        av = pools["av"].tile([64, 2, N], F32, tag="av", name="av")
        den = pools["den"].tile([64, 2, N], F32, tag="den", name="den")
        for i in range(2):  # bank i holds heads (2i, 2i+1)
            for c2 in range(2):
                he, ho = 2 * i, 2 * i + 1
                first, last = c2 == 0, c2 == 1
                # odd head first: leading-zero lhsT resets rows 0:64 on start
                nc.tensor.matmul(
                    av[0:64, i, :],
                    vt[kg][:, c2, :, i, 32:96],
                    pt[:, c2, ho, :, :],
                    start=first,
                    stop=False,
                    tile_position=(0, 0),
                    perf_mode=DR,
                    skip_group_check=True,
                )
                nc.tensor.matmul(
                    av[0:32, i, :],
                    vt[kg][:, c2, :, i, 0:32],
                    pt[:, c2, he, :, :],
                    start=False,
                    stop=last,
                    tile_position=(0, 0),
                    perf_mode=DR,
                    skip_group_check=True,
                )
                nc.tensor.matmul(
                    den[0:64, i, :],
                    W["denw"][:, :, 64:128],
                    pt[:, c2, ho, :, :],
                    start=first,
                    stop=False,
                    tile_position=(0, 0),
                    perf_mode=DR,
                    skip_group_check=True,
                )
                nc.tensor.matmul(
                    den[0:32, i, :],
                    W["denw"][:, :, 96:128],
                    pt[:, c2, he, :, :],
                    start=False,
                    stop=last,
                    tile_position=(0, 0),
                    perf_mode=DR,
                    skip_group_check=True,
                )
        rcf = pools["ar"].tile([64, 2, N], F32, tag="rcf", name="rcf")
        nc.vector.reciprocal(rcf[:, :, :], den[0:64, :, :])
        an = pools["an"].tile([64, 2, N], F16, tag="an", name="an")
        nc.vector.tensor_tensor(an[:, :, :], av[0:64, :, :], rcf[:, :, :], op=MULT)
        g, row = PAIR_DEST[p]
        wop = pools["sc"].tile([32, N], F32, tag="sc", name="sc")
        nc.tensor.matmul(
            wop[:], W["wo"][p][0][:], an[:, 0, :], start=True, stop=False
        )
        nc.tensor.matmul(
            wop[:], W["wo"][p][1][:], an[:, 1, :], start=False, stop=True
        )
        nc.vector.tensor_scalar(
            cc[g][row : row + 32, :], wop[:], W["bo"][p][:], 0.0, op0=ADD, op1=MAX
        )


def _emit_out(nc, pools, W, b, g, cc):
    wfp = pools["sc"].tile([128, N], F32, tag="sc", name="sc")
    nc.tensor.matmul(wfp[:], W["wf"][g][:], cc[g][:], start=True, stop=True)
    ot = pools["ot"].tile([128, N], F32, tag="ot", name="ot")
    nc.vector.tensor_scalar_add(ot[:], wfp[:], W["bf"][g][:])
    nc.sync.dma_start(out=W["yt_ap"][b * 2 + g], in_=ot[:])


def build_nc(n_iters: int = 1):
    """Build + compile the per-core Bass module (body repeated n_iters times)."""
    import contextlib

    n_act = sum(1 for x in ACT_SET if x)
    n_stt = 16 - n_act
    act_idx, stt_idx = {}, {}
    for i, a in enumerate(ACT_SET):
        (act_idx if a else stt_idx)[i] = len(act_idx) if a else len(stt_idx)

    nc = bacc.Bacc("TRN2", target_bir_lowering=False, debug=False)

    xt = nc.dram_tensor("xt", [BS * 2, 128, N], F16, kind="ExternalInput")
    mt8 = (
        nc.dram_tensor("mt8", [n_act, 128, 4, N], F8, kind="ExternalInput")
        if n_act
        else None
    )
    mt16 = (
        nc.dram_tensor("mt16", [n_stt, 128, 4, 1024], F16, kind="ExternalInput")
        if n_stt
        else None
    )
    wqk = nc.dram_tensor("wqk", [2, 2, 2, 128, 128], F16, kind="ExternalInput")
    wv = nc.dram_tensor("wv", [2, 128, 128], F16, kind="ExternalInput")
    bqk = nc.dram_tensor("bqk", [2, 2, 2, 128, 1], F32, kind="ExternalInput")
    bvr4 = nc.dram_tensor("bvr4", [2, 1, 512], F16, kind="ExternalInput")
    wo = nc.dram_tensor("wo", [4, 2, 64, 32], F16, kind="ExternalInput")
    bo = nc.dram_tensor("bo", [4, 32, 1], F32, kind="ExternalInput")
    wf = nc.dram_tensor("wf", [2, 64, 128], F16, kind="ExternalInput")
    bf = nc.dram_tensor("bf", [2, 128, 1], F32, kind="ExternalInput")
    onesrow = nc.dram_tensor("onesrow", [1, 128], F16, kind="ExternalInput")
    i96 = nc.dram_tensor("i96", [128, 2, 128], F8, kind="ExternalInput")
    denw = nc.dram_tensor("denw", [128, 2, 128], F8, kind="ExternalInput")
    yt = nc.dram_tensor("yt", [BS * 2, 128, N], F32, kind="ExternalOutput")

    with tile.TileContext(nc) as tc, contextlib.ExitStack() as ctx:
        pools = {
            "consts": ctx.enter_context(tc.tile_pool(name="consts", bufs=1)),
            "xt": ctx.enter_context(tc.tile_pool(name="xt", bufs=3)),
            "persist": ctx.enter_context(tc.tile_pool(name="persist", bufs=1)),
            "mt": ctx.enter_context(tc.tile_pool(name="mt", bufs=2)),
            "mt16": ctx.enter_context(tc.tile_pool(name="mt16", bufs=2)),
            "pt": ctx.enter_context(tc.tile_pool(name="pt", bufs=2)),
            "ar": ctx.enter_context(tc.tile_pool(name="ar", bufs=2)),
            "an": ctx.enter_context(tc.tile_pool(name="an", bufs=2)),
            "ot": ctx.enter_context(tc.tile_pool(name="ot", bufs=2)),
            "sc": ctx.enter_context(tc.tile_pool(name="sc", bufs=2, space="PSUM")),
            "av": ctx.enter_context(tc.tile_pool(name="av", bufs=1, space="PSUM")),
            "den": ctx.enter_context(tc.tile_pool(name="den", bufs=1, space="PSUM")),
        }
        cp = pools["consts"]
        W = {
            "xt_ap": xt.ap(),
            "mt8_ap": mt8.ap() if mt8 is not None else None,
            "mt16_ap": mt16.ap() if mt16 is not None else None,
            "mt8_idx": act_idx,
            "mt16_idx": stt_idx,
            "yt_ap": yt.ap(),
            "wqk": [
                [
                    [
                        cp.tile(
                            [128, 128], F16, tag=f"wqk{g}{qk}{t}", name=f"wqk{g}{qk}{t}"
                        )
                        for t in range(2)
                    ]
                    for qk in range(2)
                ]
                for g in range(2)
            ],
            "wv": [cp.tile([128, 128], F16, tag=f"wv{g}", name=f"wv{g}") for g in range(2)],
            "bq": [
                [cp.tile([128, 1], F32, tag=f"bq{g}{t}", name=f"bq{g}{t}") for t in range(2)]
                for g in range(2)
            ],
            "bk": [
                [cp.tile([128, 1], F32, tag=f"bk{g}{t}", name=f"bk{g}{t}") for t in range(2)]
                for g in range(2)
            ],
            "bvr4": [cp.tile([1, 512], F16, tag=f"bvr4{g}", name=f"bvr4{g}") for g in range(2)],
            "zrow16": cp.tile([1, 128], F16, tag="zrow16", name="zrow16"),
            "wo": [
                [cp.tile([64, 32], F16, tag=f"wo{p}{i}", name=f"wo{p}{i}") for i in range(2)]
                for p in range(4)
            ],
            "bo": [cp.tile([32, 1], F32, tag=f"bo{p}", name=f"bo{p}") for p in range(4)],
            "wf": [cp.tile([64, 128], F16, tag=f"wf{g}", name=f"wf{g}") for g in range(2)],
            "bf": [cp.tile([128, 1], F32, tag=f"bf{g}", name=f"bf{g}") for g in range(2)],
            "onesrow": cp.tile([1, 128], F16, tag="onesrow", name="onesrow"),
            "i96": cp.tile([128, 2, 128], F8, tag="i96", name="i96"),
            "denw": cp.tile([128, 2, 128], F8, tag="denw", name="denw"),
            "ebias": cp.tile([128, 1], F32, tag="ebias", name="ebias"),
        }
        for g in range(2):
            for qk in range(2):
                for t in range(2):
                    nc.sync.dma_start(out=W["wqk"][g][qk][t][:], in_=wqk.ap()[g, qk, t])
                    nc.sync.dma_start(
                        out=W[("bq", "bk")[qk]][g][t][:], in_=bqk.ap()[g, qk, t]
                    )
            nc.sync.dma_start(out=W["wv"][g][:], in_=wv.ap()[g])
            nc.sync.dma_start(out=W["bvr4"][g][:], in_=bvr4.ap()[g])
            nc.sync.dma_start(out=W["wf"][g][:], in_=wf.ap()[g])
            nc.sync.dma_start(out=W["bf"][g][:], in_=bf.ap()[g])
        for p in range(4):
            for i in range(2):
                nc.sync.dma_start(out=W["wo"][p][i][:], in_=wo.ap()[p, i])
            nc.sync.dma_start(out=W["bo"][p][:], in_=bo.ap()[p])
        nc.sync.dma_start(out=W["onesrow"][:], in_=onesrow.ap())
        nc.vector.memset(W["zrow16"][:], 0.0)
        nc.sync.dma_start(out=W["i96"][:, :, :], in_=i96.ap())
        nc.sync.dma_start(out=W["denw"][:, :, :], in_=denw.ap())
        nc.vector.memset(W["ebias"][:], -BIGC / SQRT_DK)

        pp = pools["persist"]
        for it in range(n_iters):
            sfx = ""
            qt = [
                [
                    pp.tile([128, 2, N], F8, tag=f"qt{b}{g}{sfx}", name=f"qt{b}{g}{sfx}")
                    for g in range(2)
                ]
                for b in range(BS)
            ]
            kt = [
                [
                    pp.tile([128, 2, N], F8, tag=f"kt{b}{g}{sfx}", name=f"kt{b}{g}{sfx}")
                    for g in range(2)
                ]
                for b in range(BS)
            ]
            vt = [
                [
                    pp.tile(
                        [128, 2, 2, 2, 96], F8, tag=f"vt{b}{g}{sfx}", name=f"vt{b}{g}{sfx}"
                    )
                    for g in range(2)
                ]
                for b in range(BS)
            ]
            cc = [
                [pp.tile([64, N], F16, tag=f"cc{b}{g}{sfx}", name=f"cc{b}{g}{sfx}") for g in range(2)]
                for b in range(BS)
            ]
            if it == 0:
                # zero the pad columns once (persist tiles are dedicated buffers)
                for b in range(BS):
                    for g in range(2):
                        nc.vector.memset(vt[b][g][:, :, :, :, 32:64], 0.0)
            # staggered emission: QKV(b+1) interleaves with attention(b)
            for g in range(2):
                _emit_qkv(nc, pools, W, 0, g, qt[0][g], kt[0][g], vt[0][g])
            for b in range(BS):
                if b + 1 < BS:
                    for g in range(2):
                        _emit_qkv(
                            nc, pools, W, b + 1, g, qt[b + 1][g], kt[b + 1][g], vt[b + 1][g]
                        )
                _emit_attn_b(nc, pools, W, b, qt[b], kt[b], vt[b], cc[b])
                for g in range(2):
                    _emit_out(nc, pools, W, b, g, cc[b])

    nc.compile()
    return nc


def _f8(x):
    import ml_dtypes

    return np.asarray(x).astype(ml_dtypes.float8_e4m3fn)


def prep_weights(inp):
    """Host-side packing of the (core-replicated) weight tensors."""
    f = np.asarray
    W = {}
    # DR col-permuted q/k weights + biases: out partition 32h+f_lo holds
    # feature 32h+16t+f_lo; cols/rows 32h+16..32h+32 zeroed.
    wqk = np.zeros((2, 2, 2, 128, 128), np.float16)
    bqk = np.zeros((2, 2, 2, 128, 1), np.float32)
    for g in range(2):
        for qk, nm in enumerate(("q", "k")):
            wsrc = f(inp[f"w{nm}{g}"])
            bsrc = f(inp[f"b{nm}{g}"])
            for t in range(2):
                for h in range(4):
                    for fl in range(16):
                        feat = 32 * h + 16 * t + fl
                        wqk[g, qk, t, :, 32 * h + fl] = wsrc[:, feat]
                        bqk[g, qk, t, 32 * h + fl, 0] = bsrc[feat]
    W["wqk"] = wqk
    W["bqk"] = bqk
    W["wv"] = np.stack([f(inp["wv0"]), f(inp["wv1"])]).astype(np.float16)
    W["bvr4"] = np.stack(
        [np.tile(f(inp["bv0"]), 4).reshape(1, 512), np.tile(f(inp["bv1"]), 4).reshape(1, 512)]
    ).astype(np.float16)
    W["wo"] = np.stack(
        [f(inp["wo00"]), f(inp["wo11"]), f(inp["wo01"]), f(inp["wo10"])]
    ).astype(np.float16).reshape(4, 2, 64, 32)
    W["bo"] = np.stack(
        [
            f(inp["bo00"]).reshape(32, 1),
            f(inp["bo11"]).reshape(32, 1),
            f(inp["bo01"]).reshape(32, 1),
            f(inp["bo10"]).reshape(32, 1),
        ]
    ).astype(np.float32)
    W["wf"] = np.stack([f(inp["wf0"]), f(inp["wf1"])]).astype(np.float16)
    W["bf"] = np.stack(
        [f(inp["bf0"]).reshape(128, 1), f(inp["bf1"]).reshape(128, 1)]
    ).astype(np.float32)
    W["onesrow"] = np.ones((1, 128), np.float16)
    i96 = np.zeros((128, 2, 128))
    i96[:, 0, :] = 96.0 * np.eye(128)
    i96[:, 1, :] = 96.0 * np.eye(128)
    W["i96"] = _f8(i96)
    denw = np.zeros((128, 2, 128))
    denw[:, :, 96:128] = 1.0
    W["denw"] = _f8(denw)
    return W


def prep_core_inputs(inp, W):
    """Build the 8 per-core in_maps (shards batch over cores)."""
    import ml_dtypes

    n_act = sum(1 for x in ACT_SET if x)
    n_stt = 16 - n_act
    x = [np.asarray(inp["x0"], np.float16), np.asarray(inp["x1"], np.float16)]
    masks = [
        np.asarray(inp["m00"]),
        np.asarray(inp["m11"]),
        np.asarray(inp["m01"]),
        np.asarray(inp["m10"]),
    ]
    in_maps = []
    for ci in range(NCORES):
        xt = np.empty((BS * 2, 128, N), np.float16)
        mt8 = np.empty((n_act, 128, 4, N), ml_dtypes.float8_e4m3fn)
        mt16 = np.empty((n_stt, 128, 4, 1024), np.float16)
        for b in range(BS):
            gb = ci * BS + b
            for g in range(2):
                xt[b * 2 + g] = x[g][gb].T
            for p in range(4):
                i = b * 4 + p
                mT = masks[p][gb].T  # [k, q]
                ch = mT.reshape(4, 128, N).transpose(1, 0, 2)  # [128, c, 512]
                if ACT_SET[i]:
                    mt8[W_act_idx(i)] = _f8(ch)
                else:
                    m3 = (MOFF + B8) * ch.astype(np.float32) - MOFF
                    mt16[W_stt_idx(i)] = np.concatenate([m3, m3], axis=2).astype(
                        np.float16
                    )
        m = {"xt": xt}
        if n_act:
            m["mt8"] = mt8
        if n_stt:
            m["mt16"] = mt16
        m.update(W)
        in_maps.append(m)
    return in_maps


def W_act_idx(i):
    return sum(1 for j in range(i) if ACT_SET[j])


def W_stt_idx(i):
    return sum(1 for j in range(i) if not ACT_SET[j])


def postprocess(results):
    """Gather per-core yt [8,128,512] -> (out0, out1) full arrays."""
    out0 = np.empty((B, N, OUT_DIM), np.float32)
    out1 = np.empty((B, N, OUT_DIM), np.float32)
    for ci in range(NCORES):
        yt = results[ci]["yt"]
        for b in range(BS):
            gb = ci * BS + b
            out0[gb] = yt[b * 2 + 0].T
            out1[gb] = yt[b * 2 + 1].T
    return out0, out1


_NC_CACHE = {}


def get_nc(n_iters: int = 1):
    if n_iters not in _NC_CACHE:
        _NC_CACHE[n_iters] = build_nc(n_iters)
    return _NC_CACHE[n_iters]


def kernel(**inputs):
    from concourse import bass_utils

    nc = get_nc(1)
    W = prep_weights(inputs)
    in_maps = prep_core_inputs(inputs, W)
    res = bass_utils.run_bass_kernel_spmd(
        nc, in_maps, core_ids=list(range(NCORES)), trace=False
    )
    return postprocess(res.results)


# revision 19
# speedup vs baseline: 1.8899x; 1.8899x over previous
"""HGAT layer Trainium2 Bass kernel.

Math (per batch element b, per group pair):
  q,k,v = relu(x @ w + b) for each group
  4 masked attentions (00, 11, 01, 10), each NH=4 heads of H=32
  inner/inter = relu(attn @ wo + bo); out_g = concat(inner_g, inter_g) @ wf_g + bf_g

Device-side design (per core, 4 batch elements, data-parallel over B=32):
  - Everything is computed in "transposed" orientation (features on SBUF
    partitions): Q^T/K^T = relu(w.T @ x^T + b), V natural [k, feat].
  - scores^T[k,q] = K_h^T.T @ Q_h^T per head, row-packed 4 heads via
    tile_position row groups (contraction = 32).
  - e = exp(scores/sqrt(dk)) on ACT (PSUM->SBUF, fp16 out),
    P^T = e * mask^T on DVE (fp16 tensor_tensor, 2x mode).
  - attn_raw^T = V_chunk.T @ P^T col-packed 4 heads (tile_position col
    groups, M=32); denominators via ones[128,32] lhsT the same way --
    gives denom broadcast over each head's 32 partitions.
  - reciprocal of denom rows batched per-b on DVE, broadcast back to 128
    partitions with a small selector matmul, normalize with one TT mul.
  - wo/wf projections stay transposed; host transposes the output back.

The masks are int32 0/1; they are host-converted to fp16 (exact) and
host-transposed/duplicated so the device reads them in the exact SBUF
layout needed ([mT_c | mT_c] per 128-row chunk, giving FD=1024 DVE ops).
"""

import sys

sys.path.insert(0, "/opt/trn_rl_repo")

import numpy as np

import concourse.bacc as bacc
import concourse.tile as tile
from concourse import mybir

B, N, NH, H = 32, 512, 4, 32
IN_DIM, OUT_DIM = 128, 128
NCORES = 8
BS = B // NCORES  # batch elements per core
SQRT_DK = float(np.sqrt(H))
F32 = mybir.dt.float32
F16 = mybir.dt.float16
ADD = mybir.AluOpType.add
MAX = mybir.AluOpType.max
MULT = mybir.AluOpType.mult
EXP = mybir.ActivationFunctionType.Exp

# pair p -> (q group, k/v group); mask m{qg}{kg}; wo{qg}{kg}
PAIRS = [(0, 0), (1, 1), (0, 1), (1, 0)]
# pair -> (out group, concat row offset): inner pairs at rows 0:32, inter at 32:64
PAIR_DEST = [(0, 0), (1, 0), (0, 32), (1, 32)]


def _emit_qkv(nc, pools, W, b, g, qt, kt, vt):
    """Emit QKV projection for (b, g). Fills qt/kt [128,512] f32, vt [128,512] f16."""
    xt_t = pools["xt"].tile([128, N], F16, tag="xt", name="xt")
    nc.sync.dma_start(out=xt_t[:], in_=W["xt_ap"][b * 2 + g])

    qp = pools["sc"].tile([128, N], F32, tag="sc", name="sc")
    nc.tensor.matmul(qp[:], W["wq"][g][:], xt_t[:], start=True, stop=True)
    nc.vector.tensor_scalar(qt[:], qp[:], W["bq"][g][:], 0.0, op0=ADD, op1=MAX)

    kp = pools["sc"].tile([128, N], F32, tag="sc", name="sc")
    nc.tensor.matmul(kp[:], W["wk"][g][:], xt_t[:], start=True, stop=True)
    nc.vector.tensor_scalar(kt[:], kp[:], W["bk"][g][:], 0.0, op0=ADD, op1=MAX)

    vp = pools["sc"].tile([128, 4, 2, 2, 32], F32, tag="sc", name="sc")
    # full-bank bias write opens the accumulation group (orders all chains)
    nc.tensor.matmul(
        vp[:, :, :, :, :], W["onesrow"][:], W["bvr4"][g][:], start=True, stop=False
    )
    for c in range(4):
        nc.tensor.matmul(
            vp[:, c, :, :, :],
            xt_t[:, 128 * c : 128 * (c + 1)],
            W["wv"][g][:],
            start=False,
            stop=False,
        )
    # full-bank +0 accumulate closes the group (runs after all chains)
    nc.tensor.matmul(vp[:, :, :, :, :], W["zrow16"][:], xt_t[0:1, :], start=False, stop=True)
    # augmented layout: even-head feats -> seg cols 0:32, odd -> 128:160
    nc.vector.tensor_scalar_max(vt[:, :, :, 0:32], vp[:, :, :, 0, :], 0.0)
    nc.vector.tensor_scalar_max(vt[:, :, :, 128:160], vp[:, :, :, 1, :], 0.0)


def _emit_attn_b(nc, pools, W, b, qt, kt, vt, cc):
    """Emit the 4 attention pairs + wo for batch element b.

    vt here is the AUGMENTED tile [128, 4(c), 2(bank), 224]: per (c, bank i)
    the 224 cols are [v_even(32) Z(32) ones(32) | Z(32) v_odd(32) Z(32) ones(32)].
    One fp16 matmul per (c, head) produces av AND broadcast den rows in the
    avden bank: rows 0:32 av_even, 32:64 av_odd, 64:96 den_even, 96:128 den_odd.
    """
    for p, (qg, kg) in enumerate(PAIRS):
        mt_t = pools["mt"].tile([128, 4 * 1024], F16, tag="mt", name="mt")
        nc.sync.dma_start(out=mt_t[:], in_=W["mt_ap"][b * 4 + p])
        avden = pools["av"].tile([128, 2, N], F32, tag="av", name="av")
        for c in range(4):
            for hh in range(2):
                sc = pools["sc"].tile([128, 1024], F32, tag="sc", name="sc")
                for j in range(2):
                    h = 2 * hh + j
                    nc.tensor.matmul(
                        sc[:, 512 * j : 512 * (j + 1)],
                        kt[kg][32 * h : 32 * (h + 1), 128 * c : 128 * (c + 1)],
                        qt[qg][32 * h : 32 * (h + 1), :],
                        start=True,
                        stop=True,
                        tile_position=(32 * h, 0),
                    )
                e = pools["e"].tile([128, 1024], F16, tag="e", name="e")
                nc.scalar.activation(e[:], sc[:], EXP, scale=1.0 / SQRT_DK)
                pt = pools["e"].tile([128, 1024], F16, tag="pt", name="pt")
                tt_eng = nc.gpsimd if (c == 0 and hh == 0) else nc.vector
                tt_eng.tensor_tensor(
                    pt[:], e[:], mt_t[:, 1024 * c : 1024 * (c + 1)], op=MULT
                )
                for j in (1, 0):
                    h = 2 * hh + j
                    i, odd = h // 2, h % 2
                    seg = vt[kg][:, c, i, 96:224] if odd else vt[kg][:, c, i, 0:96]
                    nc.tensor.matmul(
                        avden[0 : (128 if odd else 96), i, :],
                        seg,
                        pt[:, 512 * j : 512 * (j + 1)],
                        start=bool(c == 0 and odd),
                        stop=bool(c == 3 and not odd),
                        tile_position=(0, 0),
                        skip_group_check=True,
                    )
        # avden rows 64:128 hold denominators broadcast per head
        rcf = pools["ar"].tile([64, 2, N], F32, tag="rcf", name="rcf")
        nc.vector.reciprocal(rcf[:, :, :], avden[64:128, :, :])
        an = pools["an"].tile([64, 2, N], F16, tag="an", name="an")
        nc.vector.tensor_tensor(an[:, :, :], avden[0:64, :, :], rcf[:, :, :], op=MULT)
        g, row = PAIR_DEST[p]
        wop = pools["av"].tile([32, N], F32, tag="av", name="av")
        nc.tensor.matmul(wop[:], W["wo"][p][0][:], an[:, 0, :], start=True, stop=False)
        nc.tensor.matmul(wop[:], W["wo"][p][1][:], an[:, 1, :], start=False, stop=True)
        nc.vector.tensor_scalar(
            cc[g][row : row + 32, :], wop[:], W["bo"][p][:], 0.0, op0=ADD, op1=MAX
        )


def _emit_out(nc, pools, W, b, g, cc):
    wfp = pools["av"].tile([128, N], F32, tag="av", name="av")
    nc.tensor.matmul(wfp[:], W["wf"][g][:], cc[g][:], start=True, stop=True)
    ot = pools["ot"].tile([128, N], F32, tag="ot", name="ot")
    nc.vector.tensor_scalar_add(ot[:], wfp[:], W["bf"][g][:])
    nc.sync.dma_start(out=W["yt_ap"][b * 2 + g], in_=ot[:])


def build_nc(n_iters: int = 1):
    """Build + compile the per-core Bass module (body repeated n_iters times)."""
    import contextlib

    nc = bacc.Bacc("TRN2", target_bir_lowering=False, debug=False)

    xt = nc.dram_tensor("xt", [BS * 2, 128, N], F16, kind="ExternalInput")
    mt = nc.dram_tensor("mt", [BS * 4, 128, 4 * 1024], F16, kind="ExternalInput")
    wqk = nc.dram_tensor("wqk", [2, 2, 128, 128], F16, kind="ExternalInput")
    wv = nc.dram_tensor("wv", [2, 128, 128], F16, kind="ExternalInput")
    bqk = nc.dram_tensor("bqk", [2, 2, 128, 1], F32, kind="ExternalInput")
    bvr4 = nc.dram_tensor("bvr4", [2, 1, 512], F16, kind="ExternalInput")
    wo = nc.dram_tensor("wo", [4, 2, 64, 32], F16, kind="ExternalInput")
    bo = nc.dram_tensor("bo", [4, 32, 1], F32, kind="ExternalInput")
    wf = nc.dram_tensor("wf", [2, 64, 128], F16, kind="ExternalInput")
    bf = nc.dram_tensor("bf", [2, 128, 1], F32, kind="ExternalInput")
    onesrow = nc.dram_tensor("onesrow", [1, 128], F16, kind="ExternalInput")
    yt = nc.dram_tensor("yt", [BS * 2, 128, N], F32, kind="ExternalOutput")

    with tile.TileContext(nc) as tc, contextlib.ExitStack() as ctx:
        pools = {
            "consts": ctx.enter_context(tc.tile_pool(name="consts", bufs=1)),
            "xt": ctx.enter_context(tc.tile_pool(name="xt", bufs=3)),
            "persist": ctx.enter_context(tc.tile_pool(name="persist", bufs=1)),
            "mt": ctx.enter_context(tc.tile_pool(name="mt", bufs=2)),
            "e": ctx.enter_context(tc.tile_pool(name="e", bufs=6)),
            "ar": ctx.enter_context(tc.tile_pool(name="ar", bufs=5)),
            "an": ctx.enter_context(tc.tile_pool(name="an", bufs=2)),
            "ot": ctx.enter_context(tc.tile_pool(name="ot", bufs=2)),
            "sc": ctx.enter_context(tc.tile_pool(name="sc", bufs=2, space="PSUM")),
            "av": ctx.enter_context(tc.tile_pool(name="av", bufs=2, space="PSUM")),
        }
        cp = pools["consts"]
        W = {
            "xt_ap": xt.ap(),
            "mt_ap": mt.ap(),
            "yt_ap": yt.ap(),
            "wq": [cp.tile([128, 128], F16, tag=f"wq{g}", name=f"wq{g}") for g in range(2)],
            "wk": [cp.tile([128, 128], F16, tag=f"wk{g}", name=f"wk{g}") for g in range(2)],
            "wv": [cp.tile([128, 128], F16, tag=f"wv{g}", name=f"wv{g}") for g in range(2)],
            "bq": [cp.tile([128, 1], F32, tag=f"bq{g}", name=f"bq{g}") for g in range(2)],
            "bk": [cp.tile([128, 1], F32, tag=f"bk{g}", name=f"bk{g}") for g in range(2)],
            "bvr4": [cp.tile([1, 512], F16, tag=f"bvr4{g}", name=f"bvr4{g}") for g in range(2)],
            "zrow16": cp.tile([1, 128], F16, tag="zrow16", name="zrow16"),
            "wo": [
                [cp.tile([64, 32], F16, tag=f"wo{p}{i}", name=f"wo{p}{i}") for i in range(2)]
                for p in range(4)
            ],
            "bo": [cp.tile([32, 1], F32, tag=f"bo{p}", name=f"bo{p}") for p in range(4)],
            "wf": [cp.tile([64, 128], F16, tag=f"wf{g}", name=f"wf{g}") for g in range(2)],
            "bf": [cp.tile([128, 1], F32, tag=f"bf{g}", name=f"bf{g}") for g in range(2)],
            "onesrow": cp.tile([1, 128], F16, tag="onesrow", name="onesrow"),
        }
        for g in range(2):
            nc.sync.dma_start(out=W["wq"][g][:], in_=wqk.ap()[g, 0])
            nc.sync.dma_start(out=W["wk"][g][:], in_=wqk.ap()[g, 1])
            nc.sync.dma_start(out=W["wv"][g][:], in_=wv.ap()[g])
            nc.sync.dma_start(out=W["bq"][g][:], in_=bqk.ap()[g, 0])
            nc.sync.dma_start(out=W["bk"][g][:], in_=bqk.ap()[g, 1])
            nc.sync.dma_start(out=W["bvr4"][g][:], in_=bvr4.ap()[g])
            nc.sync.dma_start(out=W["wf"][g][:], in_=wf.ap()[g])
            nc.sync.dma_start(out=W["bf"][g][:], in_=bf.ap()[g])
        for p in range(4):
            for i in range(2):
                nc.sync.dma_start(out=W["wo"][p][i][:], in_=wo.ap()[p, i])
            nc.sync.dma_start(out=W["bo"][p][:], in_=bo.ap()[p])
        nc.sync.dma_start(out=W["onesrow"][:], in_=onesrow.ap())
        nc.vector.memset(W["zrow16"][:], 0.0)

        pp = pools["persist"]
        for it in range(n_iters):
            sfx = ""
            qt = [
                [pp.tile([128, N], F16, tag=f"qt{b}{g}{sfx}", name=f"qt{b}{g}{sfx}") for g in range(2)]
                for b in range(BS)
            ]
            kt = [
                [pp.tile([128, N], F16, tag=f"kt{b}{g}{sfx}", name=f"kt{b}{g}{sfx}") for g in range(2)]
                for b in range(BS)
            ]
            vt = [
                [
                    pp.tile([128, 4, 2, 224], F16, tag=f"vt{b}{g}{sfx}", name=f"vt{b}{g}{sfx}")
                    for g in range(2)
                ]
                for b in range(BS)
            ]
            cc = [
                [pp.tile([64, N], F16, tag=f"cc{b}{g}{sfx}", name=f"cc{b}{g}{sfx}") for g in range(2)]
                for b in range(BS)
            ]
            if it == 0:
                for b in range(BS):
                    for g in range(2):
                        nc.vector.memset(vt[b][g][:, :, :, 32:64], 0.0)
                        nc.vector.memset(vt[b][g][:, :, :, 96:128], 0.0)
                        nc.vector.memset(vt[b][g][:, :, :, 160:192], 0.0)
                        nc.vector.memset(vt[b][g][:, :, :, 64:96], 1.0)
                        nc.vector.memset(vt[b][g][:, :, :, 192:224], 1.0)
            # staggered emission: QKV(b+1) interleaves with attention(b)
            for g in range(2):
                _emit_qkv(nc, pools, W, 0, g, qt[0][g], kt[0][g], vt[0][g])
            for b in range(BS):
                if b + 1 < BS:
                    for g in range(2):
                        _emit_qkv(
                            nc, pools, W, b + 1, g, qt[b + 1][g], kt[b + 1][g], vt[b + 1][g]
                        )
                _emit_attn_b(nc, pools, W, b, qt[b], kt[b], vt[b], cc[b])
                for g in range(2):
                    _emit_out(nc, pools, W, b, g, cc[b])

    nc.compile()
    return nc


def prep_weights(inp):
    """Host-side packing of the (core-replicated) weight tensors."""
    f = np.asarray
    W = {}
    W["wqk"] = np.stack(
        [
            np.stack([f(inp["wq0"]), f(inp["wk0"])]),
            np.stack([f(inp["wq1"]), f(inp["wk1"])]),
        ]
    ).astype(np.float16)
    W["wv"] = np.stack([f(inp["wv0"]), f(inp["wv1"])]).astype(np.float16)
    W["bqk"] = np.stack(
        [
            np.stack([f(inp["bq0"]).reshape(128, 1), f(inp["bk0"]).reshape(128, 1)]),
            np.stack([f(inp["bq1"]).reshape(128, 1), f(inp["bk1"]).reshape(128, 1)]),
        ]
    ).astype(np.float32)
    W["bvr4"] = np.stack(
        [np.tile(f(inp["bv0"]), 4).reshape(1, 512), np.tile(f(inp["bv1"]), 4).reshape(1, 512)]
    ).astype(np.float16)
    W["wo"] = np.stack(
        [f(inp["wo00"]), f(inp["wo11"]), f(inp["wo01"]), f(inp["wo10"])]
    ).astype(np.float16).reshape(4, 2, 64, 32)
    W["bo"] = np.stack(
        [
            f(inp["bo00"]).reshape(32, 1),
            f(inp["bo11"]).reshape(32, 1),
            f(inp["bo01"]).reshape(32, 1),
            f(inp["bo10"]).reshape(32, 1),
        ]
    ).astype(np.float32)
    W["wf"] = np.stack([f(inp["wf0"]), f(inp["wf1"])]).astype(np.float16)
    W["bf"] = np.stack(
        [f(inp["bf0"]).reshape(128, 1), f(inp["bf1"]).reshape(128, 1)]
    ).astype(np.float32)
    W["onesrow"] = np.ones((1, 128), np.float16)
    return W


def prep_core_inputs(inp, W):
    """Build the 8 per-core in_maps (shards batch over cores)."""
    x = [np.asarray(inp["x0"], np.float32), np.asarray(inp["x1"], np.float32)]
    masks = [
        np.asarray(inp["m00"]),
        np.asarray(inp["m11"]),
        np.asarray(inp["m01"]),
        np.asarray(inp["m10"]),
    ]
    in_maps = []
    for ci in range(NCORES):
        xt = np.empty((BS * 2, 128, N), np.float16)
        mt = np.empty((BS * 4, 128, 4 * 1024), np.float16)
        for b in range(BS):
            gb = ci * BS + b
            for g in range(2):
                xt[b * 2 + g] = x[g][gb].T
            for p in range(4):
                mT = masks[p][gb].T.astype(np.float16)  # [k, q]
                ch = mT.reshape(4, 128, N)  # chunk c = k rows 128c..
                dup = np.stack([ch, ch], axis=1)  # [4, 2, 128, N]
                mt[b * 4 + p] = dup.transpose(2, 0, 1, 3).reshape(128, 4 * 1024)
        m = {"xt": xt, "mt": mt}
        m.update(W)
        in_maps.append(m)
    return in_maps


def postprocess(results):
    """Gather per-core yt [8,128,512] -> (out0, out1) full arrays."""
    out0 = np.empty((B, N, OUT_DIM), np.float32)
    out1 = np.empty((B, N, OUT_DIM), np.float32)
    for ci in range(NCORES):
        yt = results[ci]["yt"]
        for b in range(BS):
            gb = ci * BS + b
            out0[gb] = yt[b * 2 + 0].T
            out1[gb] = yt[b * 2 + 1].T
    return out0, out1


_NC_CACHE = {}


def get_nc(n_iters: int = 1):
    if n_iters not in _NC_CACHE:
        _NC_CACHE[n_iters] = build_nc(n_iters)
    return _NC_CACHE[n_iters]


def kernel(**inputs):
    from concourse import bass_utils

    nc = get_nc(1)
    W = prep_weights(inputs)
    in_maps = prep_core_inputs(inputs, W)
    res = bass_utils.run_bass_kernel_spmd(
        nc, in_maps, core_ids=list(range(NCORES)), trace=False
    )
    return postprocess(res.results)



# revision 22
# speedup vs baseline: 8.1979x; 4.3377x over previous
"""HGAT layer Trainium2 Bass kernel.

Math (per batch element b, per group pair):
  q,k,v = relu(x @ w + b) for each group
  4 masked attentions (00, 11, 01, 10), each NH=4 heads of H=32
  inner/inter = relu(attn @ wo + bo); out_g = concat(inner_g, inter_g) @ wf_g + bf_g

Device-side design (per core, 4 batch elements, data-parallel over B=32):
  - Everything is computed in "transposed" orientation (features on SBUF
    partitions): Q^T/K^T = relu(w.T @ x^T + b), V natural [k, feat].
  - scores^T[k,q] = K_h^T.T @ Q_h^T per head, row-packed 4 heads via
    tile_position row groups (contraction = 32).
  - e = exp(scores/sqrt(dk)) on ACT (PSUM->SBUF, fp16 out),
    P^T = e * mask^T on DVE (fp16 tensor_tensor, 2x mode).
  - attn_raw^T = V_chunk.T @ P^T col-packed 4 heads (tile_position col
    groups, M=32); denominators via ones[128,32] lhsT the same way --
    gives denom broadcast over each head's 32 partitions.
  - reciprocal of denom rows batched per-b on DVE, broadcast back to 128
    partitions with a small selector matmul, normalize with one TT mul.
  - wo/wf projections stay transposed; host transposes the output back.

The masks are int32 0/1; they are host-converted to fp16 (exact) and
host-transposed/duplicated so the device reads them in the exact SBUF
layout needed ([mT_c | mT_c] per 128-row chunk, giving FD=1024 DVE ops).
"""

import sys

sys.path.insert(0, "/opt/trn_rl_repo")

import numpy as np

import concourse.bacc as bacc
import concourse.tile as tile
from concourse import mybir

B, N, NH, H = 32, 512, 4, 32
IN_DIM, OUT_DIM = 128, 128
NCORES = 8
BS = B // NCORES  # batch elements per core
SQRT_DK = float(np.sqrt(H))
F32 = mybir.dt.float32
F16 = mybir.dt.float16
ADD = mybir.AluOpType.add
MAX = mybir.AluOpType.max
MULT = mybir.AluOpType.mult
EXP = mybir.ActivationFunctionType.Exp

# pair p -> (q group, k/v group); mask m{qg}{kg}; wo{qg}{kg}
PAIRS = [(0, 0), (1, 1), (0, 1), (1, 0)]
# pair -> (out group, concat row offset): inner pairs at rows 0:32, inter at 32:64
PAIR_DEST = [(0, 0), (1, 0), (0, 32), (1, 32)]


def _emit_qkv(nc, pools, W, b, g, qt, kt, vt):
    """Emit QKV projection for (b, g). Fills qt/kt [128,512] f32, vt [128,512] f16."""
    xt_t = pools["xt"].tile([128, N], F16, tag="xt", name="xt")
    nc.sync.dma_start(out=xt_t[:], in_=W["xt_ap"][b * 2 + g])

    qp = pools["sc"].tile([128, N], F32, tag="sc", name="sc")
    nc.tensor.matmul(qp[:], W["wq"][g][:], xt_t[:], start=True, stop=True)
    nc.vector.tensor_scalar(qt[:], qp[:], W["bq"][g][:], 0.0, op0=ADD, op1=MAX)

    kp = pools["sc"].tile([128, N], F32, tag="sc", name="sc")
    nc.tensor.matmul(kp[:], W["wk"][g][:], xt_t[:], start=True, stop=True)
    nc.vector.tensor_scalar(kt[:], kp[:], W["bk"][g][:], 0.0, op0=ADD, op1=MAX)

    vp = pools["sc"].tile([128, 4, 2, 2, 32], F32, tag="sc", name="sc")
    # full-bank bias write opens the accumulation group (orders all chains)
    nc.tensor.matmul(
        vp[:, :, :, :, :], W["onesrow"][:], W["bvr4"][g][:], start=True, stop=False
    )
    for c in range(4):
        nc.tensor.matmul(
            vp[:, c, :, :, :],
            xt_t[:, 128 * c : 128 * (c + 1)],
            W["wv"][g][:],
            start=False,
            stop=False,
        )
    # full-bank +0 accumulate closes the group (runs after all chains)
    nc.tensor.matmul(vp[:, :, :, :, :], W["zrow16"][:], xt_t[0:1, :], start=False, stop=True)
    # augmented layout: even-head feats -> seg cols 0:32, odd -> 128:160
    nc.vector.tensor_scalar_max(vt[:, :, :, 0:32], vp[:, :, :, 0, :], 0.0)
    nc.vector.tensor_scalar_max(vt[:, :, :, 128:160], vp[:, :, :, 1, :], 0.0)


def _emit_attn_b(nc, pools, W, b, qt, kt, vt, cc):
    """Emit the 4 attention pairs + wo for batch element b.

    vt here is the AUGMENTED tile [128, 4(c), 2(bank), 224]: per (c, bank i)
    the 224 cols are [v_even(32) Z(32) ones(32) | Z(32) v_odd(32) Z(32) ones(32)].
    One fp16 matmul per (c, head) produces av AND broadcast den rows in the
    avden bank: rows 0:32 av_even, 32:64 av_odd, 64:96 den_even, 96:128 den_odd.
    """
    for p, (qg, kg) in enumerate(PAIRS):
        mt_t = pools["mt"].tile([128, 4 * 1024], F16, tag="mt", name="mt")
        nc.sync.dma_start(out=mt_t[:], in_=W["mt_ap"][b * 4 + p])
        avden = pools["av"].tile([128, 2, N], F32, tag="av", name="av")
        pts = {}
        for c in range(4):
            for hh in range(2):
                sc = pools["sc"].tile([128, 1024], F32, tag="sc", name="sc")
                for j in range(2):
                    h = 2 * hh + j
                    nc.tensor.matmul(
                        sc[:, 512 * j : 512 * (j + 1)],
                        kt[kg][32 * h : 32 * (h + 1), 128 * c : 128 * (c + 1)],
                        qt[qg][32 * h : 32 * (h + 1), :],
                        start=True,
                        stop=True,
                        tile_position=(32 * h, 0),
                    )
                e = pools["e"].tile([128, 1024], F16, tag="e", name="e")
                nc.scalar.activation(e[:], sc[:], EXP, scale=1.0 / SQRT_DK)
                pt = pools["e"].tile([128, 1024], F16, tag="pt", name="pt")
                # chunk-0 mask mults on Pool: their av matmuls run last in the
                # reordered chain, so Pool latency is hidden
                tt_eng = nc.gpsimd if c == 0 else nc.vector
                tt_eng.tensor_tensor(
                    pt[:], e[:], mt_t[:, 1024 * c : 1024 * (c + 1)], op=MULT
                )
                pts[(c, hh)] = pt
        # av/den accumulation in chunk order 1,2,3,0 (start on 1, stop on 0)
        for ci, c in enumerate((1, 2, 3, 0)):
            for hh in range(2):
                pt = pts[(c, hh)]
                for j in (1, 0):
                    h = 2 * hh + j
                    i, odd = h // 2, h % 2
                    seg = vt[kg][:, c, i, 96:224] if odd else vt[kg][:, c, i, 0:96]
                    nc.tensor.matmul(
                        avden[0 : (128 if odd else 96), i, :],
                        seg,
                        pt[:, 512 * j : 512 * (j + 1)],
                        start=bool(ci == 0 and odd),
                        stop=bool(ci == 3 and not odd),
                        tile_position=(0, 0),
                        skip_group_check=True,
                    )
        # avden rows 64:128 hold denominators broadcast per head
        rcf = pools["ar"].tile([64, 2, N], F32, tag="rcf", name="rcf")
        nc.vector.reciprocal(rcf[:, :, :], avden[64:128, :, :])
        an = pools["an"].tile([64, 2, N], F16, tag="an", name="an")
        nc.vector.tensor_tensor(an[:, :, :], avden[0:64, :, :], rcf[:, :, :], op=MULT)
        g, row = PAIR_DEST[p]
        wop = pools["av"].tile([32, N], F32, tag="av", name="av")
        nc.tensor.matmul(wop[:], W["wo"][p][0][:], an[:, 0, :], start=True, stop=False)
        nc.tensor.matmul(wop[:], W["wo"][p][1][:], an[:, 1, :], start=False, stop=True)
        nc.vector.tensor_scalar(
            cc[g][row : row + 32, :], wop[:], W["bo"][p][:], 0.0, op0=ADD, op1=MAX
        )


def _emit_out(nc, pools, W, b, g, cc):
    wfp = pools["av"].tile([128, N], F32, tag="av", name="av")
    nc.tensor.matmul(wfp[:], W["wf"][g][:], cc[g][:], start=True, stop=True)
    ot = pools["ot"].tile([128, N], F32, tag="ot", name="ot")
    nc.vector.tensor_scalar_add(ot[:], wfp[:], W["bf"][g][:])
    nc.sync.dma_start(out=W["yt_ap"][b * 2 + g], in_=ot[:])


def build_nc(n_iters: int = 1):
    """Build + compile the per-core Bass module (body repeated n_iters times)."""
    import contextlib

    nc = bacc.Bacc("TRN2", target_bir_lowering=False, debug=False)

    xt = nc.dram_tensor("xt", [BS * 2, 128, N], F16, kind="ExternalInput")
    mt = nc.dram_tensor("mt", [BS * 4, 128, 4 * 1024], F16, kind="ExternalInput")
    wqk = nc.dram_tensor("wqk", [2, 2, 128, 128], F16, kind="ExternalInput")
    wv = nc.dram_tensor("wv", [2, 128, 128], F16, kind="ExternalInput")
    bqk = nc.dram_tensor("bqk", [2, 2, 128, 1], F32, kind="ExternalInput")
    bvr4 = nc.dram_tensor("bvr4", [2, 1, 512], F16, kind="ExternalInput")
    wo = nc.dram_tensor("wo", [4, 2, 64, 32], F16, kind="ExternalInput")
    bo = nc.dram_tensor("bo", [4, 32, 1], F32, kind="ExternalInput")
    wf = nc.dram_tensor("wf", [2, 64, 128], F16, kind="ExternalInput")
    bf = nc.dram_tensor("bf", [2, 128, 1], F32, kind="ExternalInput")
    onesrow = nc.dram_tensor("onesrow", [1, 128], F16, kind="ExternalInput")
    yt = nc.dram_tensor("yt", [BS * 2, 128, N], F32, kind="ExternalOutput")

    with tile.TileContext(nc) as tc, contextlib.ExitStack() as ctx:
        pools = {
            "consts": ctx.enter_context(tc.tile_pool(name="consts", bufs=1)),
            "xt": ctx.enter_context(tc.tile_pool(name="xt", bufs=3)),
            "persist": ctx.enter_context(tc.tile_pool(name="persist", bufs=1)),
            "mt": ctx.enter_context(tc.tile_pool(name="mt", bufs=3)),
            "e": ctx.enter_context(tc.tile_pool(name="e", bufs=10)),
            "ar": ctx.enter_context(tc.tile_pool(name="ar", bufs=5)),
            "an": ctx.enter_context(tc.tile_pool(name="an", bufs=2)),
            "ot": ctx.enter_context(tc.tile_pool(name="ot", bufs=2)),
            "sc": ctx.enter_context(tc.tile_pool(name="sc", bufs=2, space="PSUM")),
            "av": ctx.enter_context(tc.tile_pool(name="av", bufs=2, space="PSUM")),
        }
        cp = pools["consts"]
        W = {
            "xt_ap": xt.ap(),
            "mt_ap": mt.ap(),
            "yt_ap": yt.ap(),
            "wq": [cp.tile([128, 128], F16, tag=f"wq{g}", name=f"wq{g}") for g in range(2)],
            "wk": [cp.tile([128, 128], F16, tag=f"wk{g}", name=f"wk{g}") for g in range(2)],
            "wv": [cp.tile([128, 128], F16, tag=f"wv{g}", name=f"wv{g}") for g in range(2)],
            "bq": [cp.tile([128, 1], F32, tag=f"bq{g}", name=f"bq{g}") for g in range(2)],
            "bk": [cp.tile([128, 1], F32, tag=f"bk{g}", name=f"bk{g}") for g in range(2)],
            "bvr4": [cp.tile([1, 512], F16, tag=f"bvr4{g}", name=f"bvr4{g}") for g in range(2)],
            "zrow16": cp.tile([1, 128], F16, tag="zrow16", name="zrow16"),
            "wo": [
                [cp.tile([64, 32], F16, tag=f"wo{p}{i}", name=f"wo{p}{i}") for i in range(2)]
                for p in range(4)
            ],
            "bo": [cp.tile([32, 1], F32, tag=f"bo{p}", name=f"bo{p}") for p in range(4)],
            "wf": [cp.tile([64, 128], F16, tag=f"wf{g}", name=f"wf{g}") for g in range(2)],
            "bf": [cp.tile([128, 1], F32, tag=f"bf{g}", name=f"bf{g}") for g in range(2)],
            "onesrow": cp.tile([1, 128], F16, tag="onesrow", name="onesrow"),
        }
        for g in range(2):
            nc.sync.dma_start(out=W["wq"][g][:], in_=wqk.ap()[g, 0])
            nc.sync.dma_start(out=W["wk"][g][:], in_=wqk.ap()[g, 1])
            nc.sync.dma_start(out=W["wv"][g][:], in_=wv.ap()[g])
            nc.sync.dma_start(out=W["bq"][g][:], in_=bqk.ap()[g, 0])
            nc.sync.dma_start(out=W["bk"][g][:], in_=bqk.ap()[g, 1])
            nc.sync.dma_start(out=W["bvr4"][g][:], in_=bvr4.ap()[g])
            nc.sync.dma_start(out=W["wf"][g][:], in_=wf.ap()[g])
            nc.sync.dma_start(out=W["bf"][g][:], in_=bf.ap()[g])
        for p in range(4):
            for i in range(2):
                nc.sync.dma_start(out=W["wo"][p][i][:], in_=wo.ap()[p, i])
            nc.sync.dma_start(out=W["bo"][p][:], in_=bo.ap()[p])
        nc.sync.dma_start(out=W["onesrow"][:], in_=onesrow.ap())
        nc.vector.memset(W["zrow16"][:], 0.0)

        pp = pools["persist"]
        for it in range(n_iters):
            sfx = ""
            qt = [
                [pp.tile([128, N], F16, tag=f"qt{b}{g}{sfx}", name=f"qt{b}{g}{sfx}") for g in range(2)]
                for b in range(BS)
            ]
            kt = [
                [pp.tile([128, N], F16, tag=f"kt{b}{g}{sfx}", name=f"kt{b}{g}{sfx}") for g in range(2)]
                for b in range(BS)
            ]
            vt = [
                [
                    pp.tile([128, 4, 2, 224], F16, tag=f"vt{b}{g}{sfx}", name=f"vt{b}{g}{sfx}")
                    for g in range(2)
                ]
                for b in range(BS)
            ]
            cc = [
                [pp.tile([64, N], F16, tag=f"cc{b}{g}{sfx}", name=f"cc{b}{g}{sfx}") for g in range(2)]
                for b in range(BS)
            ]
            if it == 0:
                for b in range(BS):
                    for g in range(2):
                        nc.vector.memset(vt[b][g][:, :, :, 32:64], 0.0)
                        nc.vector.memset(vt[b][g][:, :, :, 96:128], 0.0)
                        nc.vector.memset(vt[b][g][:, :, :, 160:192], 0.0)
                        nc.vector.memset(vt[b][g][:, :, :, 64:96], 1.0)
                        nc.vector.memset(vt[b][g][:, :, :, 192:224], 1.0)
            # staggered emission: QKV(b+1) interleaves with attention(b)
            for g in range(2):
                _emit_qkv(nc, pools, W, 0, g, qt[0][g], kt[0][g], vt[0][g])
            for b in range(BS):
                if b + 1 < BS:
                    for g in range(2):
                        _emit_qkv(
                            nc, pools, W, b + 1, g, qt[b + 1][g], kt[b + 1][g], vt[b + 1][g]
                        )
                _emit_attn_b(nc, pools, W, b, qt[b], kt[b], vt[b], cc[b])
                for g in range(2):
                    _emit_out(nc, pools, W, b, g, cc[b])

    nc.compile()
    return nc


def prep_weights(inp):
    """Host-side packing of the (core-replicated) weight tensors."""
    f = np.asarray
    W = {}
    W["wqk"] = np.stack(
        [
            np.stack([f(inp["wq0"]), f(inp["wk0"])]),
            np.stack([f(inp["wq1"]), f(inp["wk1"])]),
        ]
    ).astype(np.float16)
    W["wv"] = np.stack([f(inp["wv0"]), f(inp["wv1"])]).astype(np.float16)
    W["bqk"] = np.stack(
        [
            np.stack([f(inp["bq0"]).reshape(128, 1), f(inp["bk0"]).reshape(128, 1)]),
            np.stack([f(inp["bq1"]).reshape(128, 1), f(inp["bk1"]).reshape(128, 1)]),
        ]
    ).astype(np.float32)
    W["bvr4"] = np.stack(
        [np.tile(f(inp["bv0"]), 4).reshape(1, 512), np.tile(f(inp["bv1"]), 4).reshape(1, 512)]
    ).astype(np.float16)
    W["wo"] = np.stack(
        [f(inp["wo00"]), f(inp["wo11"]), f(inp["wo01"]), f(inp["wo10"])]
    ).astype(np.float16).reshape(4, 2, 64, 32)
    W["bo"] = np.stack(
        [
            f(inp["bo00"]).reshape(32, 1),
            f(inp["bo11"]).reshape(32, 1),
            f(inp["bo01"]).reshape(32, 1),
            f(inp["bo10"]).reshape(32, 1),
        ]
    ).astype(np.float32)
    W["wf"] = np.stack([f(inp["wf0"]), f(inp["wf1"])]).astype(np.float16)
    W["bf"] = np.stack(
        [f(inp["bf0"]).reshape(128, 1), f(inp["bf1"]).reshape(128, 1)]
    ).astype(np.float32)
    W["onesrow"] = np.ones((1, 128), np.float16)
    return W


def prep_core_inputs(inp, W):
    """Build the 8 per-core in_maps (shards batch over cores)."""
    x = [np.asarray(inp["x0"], np.float32), np.asarray(inp["x1"], np.float32)]
    masks = [
        np.asarray(inp["m00"]),
        np.asarray(inp["m11"]),
        np.asarray(inp["m01"]),
        np.asarray(inp["m10"]),
    ]
    in_maps = []
    for ci in range(NCORES):
        xt = np.empty((BS * 2, 128, N), np.float16)
        mt = np.empty((BS * 4, 128, 4 * 1024), np.float16)
        for b in range(BS):
            gb = ci * BS + b
            for g in range(2):
                xt[b * 2 + g] = x[g][gb].T
            for p in range(4):
                mT = masks[p][gb].T.astype(np.float16)  # [k, q]
                ch = mT.reshape(4, 128, N)  # chunk c = k rows 128c..
                dup = np.stack([ch, ch], axis=1)  # [4, 2, 128, N]
                mt[b * 4 + p] = dup.transpose(2, 0, 1, 3).reshape(128, 4 * 1024)
        m = {"xt": xt, "mt": mt}
        m.update(W)
        in_maps.append(m)
    return in_maps


def postprocess(results):
    """Gather per-core yt [8,128,512] -> (out0, out1) full arrays."""
    out0 = np.empty((B, N, OUT_DIM), np.float32)
    out1 = np.empty((B, N, OUT_DIM), np.float32)
    for ci in range(NCORES):
        yt = results[ci]["yt"]
        for b in range(BS):
            gb = ci * BS + b
            out0[gb] = yt[b * 2 + 0].T
            out1[gb] = yt[b * 2 + 1].T
    return out0, out1


_NC_CACHE = {}


def get_nc(n_iters: int = 1):
    if n_iters not in _NC_CACHE:
        _NC_CACHE[n_iters] = build_nc(n_iters)
    return _NC_CACHE[n_iters]


def kernel(**inputs):
    from concourse import bass_utils

    nc = get_nc(1)
    W = prep_weights(inputs)
    in_maps = prep_core_inputs(inputs, W)
    res = bass_utils.run_bass_kernel_spmd(
        nc, in_maps, core_ids=list(range(NCORES)), trace=False
    )
    return postprocess(res.results)



# revision 25
# speedup vs baseline: 26.6880x; 3.2555x over previous
"""HGAT layer Trainium2 Bass kernel.

Math (per batch element b, per group pair):
  q,k,v = relu(x @ w + b) for each group
  4 masked attentions (00, 11, 01, 10), each NH=4 heads of H=32
  inner/inter = relu(attn @ wo + bo); out_g = concat(inner_g, inter_g) @ wf_g + bf_g

Device-side design (per core, 4 batch elements, data-parallel over B=32):
  - Everything is computed in "transposed" orientation (features on SBUF
    partitions): Q^T/K^T = relu(w.T @ x^T + b), V natural [k, feat].
  - scores^T[k,q] = K_h^T.T @ Q_h^T per head, row-packed 4 heads via
    tile_position row groups (contraction = 32).
  - e = exp(scores/sqrt(dk)) on ACT (PSUM->SBUF, fp16 out),
    P^T = e * mask^T on DVE (fp16 tensor_tensor, 2x mode).
  - attn_raw^T = V_chunk.T @ P^T col-packed 4 heads (tile_position col
    groups, M=32); denominators via ones[128,32] lhsT the same way --
    gives denom broadcast over each head's 32 partitions.
  - reciprocal of denom rows batched per-b on DVE, broadcast back to 128
    partitions with a small selector matmul, normalize with one TT mul.
  - wo/wf projections stay transposed; host transposes the output back.

The masks are int32 0/1; they are host-converted to fp16 (exact) and
host-transposed/duplicated so the device reads them in the exact SBUF
layout needed ([mT_c | mT_c] per 128-row chunk, giving FD=1024 DVE ops).
"""

import sys

sys.path.insert(0, "/opt/trn_rl_repo")

import numpy as np

import concourse.bacc as bacc
import concourse.tile as tile
from concourse import mybir

B, N, NH, H = 32, 512, 4, 32
IN_DIM, OUT_DIM = 128, 128
NCORES = 8
BS = B // NCORES  # batch elements per core
SQRT_DK = float(np.sqrt(H))
F32 = mybir.dt.float32
F16 = mybir.dt.float16
ADD = mybir.AluOpType.add
MAX = mybir.AluOpType.max
MULT = mybir.AluOpType.mult
EXP = mybir.ActivationFunctionType.Exp

# pair p -> (q group, k/v group); mask m{qg}{kg}; wo{qg}{kg}
PAIRS = [(0, 0), (1, 1), (0, 1), (1, 0)]
# pair -> (out group, concat row offset): inner pairs at rows 0:32, inter at 32:64
PAIR_DEST = [(0, 0), (1, 0), (0, 32), (1, 32)]


def _emit_qkv(nc, pools, W, b, g, qt, kt, vt):
    """Emit QKV projection for (b, g). Fills qt/kt [128,512] f32, vt [128,512] f16."""
    xt_t = pools["xt"].tile([128, N], F16, tag="xt", name="xt")
    nc.sync.dma_start(out=xt_t[:], in_=W["xt_ap"][b * 2 + g])

    qp = pools["sc"].tile([128, N], F32, tag="sc", name="sc")
    nc.tensor.matmul(qp[:], W["wq"][g][:], xt_t[:], start=True, stop=True)
    nc.vector.tensor_scalar(qt[:], qp[:], W["bq"][g][:], 0.0, op0=ADD, op1=MAX)

    kp = pools["sc"].tile([128, N], F32, tag="sc", name="sc")
    nc.tensor.matmul(kp[:], W["wk"][g][:], xt_t[:], start=True, stop=True)
    nc.vector.tensor_scalar(kt[:], kp[:], W["bk"][g][:], 0.0, op0=ADD, op1=MAX)

    vp = pools["sc"].tile([128, 4, 2, 2, 32], F32, tag="sc", name="sc")
    # full-bank bias write opens the accumulation group (orders all chains)
    nc.tensor.matmul(
        vp[:, :, :, :, :], W["onesrow"][:], W["bvr4"][g][:], start=True, stop=False
    )
    for c in range(4):
        nc.tensor.matmul(
            vp[:, c, :, :, :],
            xt_t[:, 128 * c : 128 * (c + 1)],
            W["wv"][g][:],
            start=False,
            stop=False,
        )
    # full-bank +0 accumulate closes the group (runs after all chains)
    nc.tensor.matmul(vp[:, :, :, :, :], W["zrow16"][:], xt_t[0:1, :], start=False, stop=True)
    # augmented layout: even-head feats -> seg cols 0:32, odd -> 128:160
    nc.vector.tensor_scalar_max(vt[:, :, :, 0:32], vp[:, :, :, 0, :], 0.0)
    nc.vector.tensor_scalar_max(vt[:, :, :, 128:160], vp[:, :, :, 1, :], 0.0)


def _emit_attn_p(nc, pools, W, b, p, qt, kt, vt, cc):
    """Emit one attention pair + wo for batch element b.

    vt here is the AUGMENTED tile [128, 4(c), 2(bank), 224]: per (c, bank i)
    the 224 cols are [v_even(32) Z(32) ones(32) | Z(32) v_odd(32) Z(32) ones(32)].
    One fp16 matmul per (c, head) produces av AND broadcast den rows in the
    avden bank: rows 0:32 av_even, 32:64 av_odd, 64:96 den_even, 96:128 den_odd.
    """
    if True:
        qg, kg = PAIRS[p]
        mt_t = pools["mt"].tile([128, 4 * 1024], F16, tag="mt", name="mt")
        nc.sync.dma_start(out=mt_t[:], in_=W["mt_ap"][b * 4 + p])
        avden = pools["av"].tile([128, 2, N], F32, tag="av", name="av")
        pts = {}
        for c in range(4):
            for hh in range(2):
                sc = pools["sc"].tile([128, 1024], F32, tag="sc", name="sc")
                for j in range(2):
                    h = 2 * hh + j
                    nc.tensor.matmul(
                        sc[:, 512 * j : 512 * (j + 1)],
                        kt[kg][32 * h : 32 * (h + 1), 128 * c : 128 * (c + 1)],
                        qt[qg][32 * h : 32 * (h + 1), :],
                        start=True,
                        stop=True,
                        tile_position=(32 * h, 0),
                    )
                e = pools["e"].tile([128, 1024], F16, tag="e", name="e")
                nc.scalar.activation(e[:], sc[:], EXP, scale=1.0 / SQRT_DK)
                pt = pools["e"].tile([128, 1024], F16, tag="pt", name="pt")
                # chunk-0 mask mults on Pool: their av matmuls run last in the
                # reordered chain, so Pool latency is hidden
                tt_eng = nc.gpsimd if c == 0 else nc.vector
                tt_eng.tensor_tensor(
                    pt[:], e[:], mt_t[:, 1024 * c : 1024 * (c + 1)], op=MULT
                )
                pts[(c, hh)] = pt
        # av/den accumulation in chunk order 1,2,3,0 (start on 1, stop on 0)
        for ci, c in enumerate((1, 2, 3, 0)):
            for hh in range(2):
                pt = pts[(c, hh)]
                for j in (1, 0):
                    h = 2 * hh + j
                    i, odd = h // 2, h % 2
                    seg = vt[kg][:, c, i, 96:224] if odd else vt[kg][:, c, i, 0:96]
                    nc.tensor.matmul(
                        avden[0 : (128 if odd else 96), i, :],
                        seg,
                        pt[:, 512 * j : 512 * (j + 1)],
                        start=bool(ci == 0 and odd),
                        stop=bool(ci == 3 and not odd),
                        tile_position=(0, 0),
                        skip_group_check=True,
                    )
        # avden rows 64:128 hold denominators broadcast per head
        rcf = pools["ar"].tile([64, 2, N], F32, tag="rcf", name="rcf")
        nc.vector.reciprocal(rcf[:, :, :], avden[64:128, :, :])
        an = pools["an"].tile([64, 2, N], F16, tag="an", name="an")
        nc.vector.tensor_tensor(an[:, :, :], avden[0:64, :, :], rcf[:, :, :], op=MULT)
        g, row = PAIR_DEST[p]
        wop = pools["av"].tile([32, N], F32, tag="av", name="av")
        nc.tensor.matmul(wop[:], W["wo"][p][0][:], an[:, 0, :], start=True, stop=False)
        nc.tensor.matmul(wop[:], W["wo"][p][1][:], an[:, 1, :], start=False, stop=True)
        nc.vector.tensor_scalar(
            cc[g][row : row + 32, :], wop[:], W["bo"][p][:], 0.0, op0=ADD, op1=MAX
        )


def _emit_out(nc, pools, W, b, g, cc):
    wfp = pools["av"].tile([128, N], F32, tag="av", name="av")
    nc.tensor.matmul(wfp[:], W["wf"][g][:], cc[g][:], start=True, stop=True)
    ot = pools["ot"].tile([128, N], F32, tag="ot", name="ot")
    nc.vector.tensor_scalar_add(ot[:], wfp[:], W["bf"][g][:])
    nc.sync.dma_start(out=W["yt_ap"][b * 2 + g], in_=ot[:])


def build_nc(n_iters: int = 1):
    """Build + compile the per-core Bass module (body repeated n_iters times)."""
    import contextlib

    nc = bacc.Bacc("TRN2", target_bir_lowering=False, debug=False)

    xt = nc.dram_tensor("xt", [BS * 2, 128, N], F16, kind="ExternalInput")
    mt = nc.dram_tensor("mt", [BS * 4, 128, 4 * 1024], F16, kind="ExternalInput")
    wqk = nc.dram_tensor("wqk", [2, 2, 128, 128], F16, kind="ExternalInput")
    wv = nc.dram_tensor("wv", [2, 128, 128], F16, kind="ExternalInput")
    bqk = nc.dram_tensor("bqk", [2, 2, 128, 1], F32, kind="ExternalInput")
    bvr4 = nc.dram_tensor("bvr4", [2, 1, 512], F16, kind="ExternalInput")
    wo = nc.dram_tensor("wo", [4, 2, 64, 32], F16, kind="ExternalInput")
    bo = nc.dram_tensor("bo", [4, 32, 1], F32, kind="ExternalInput")
    wf = nc.dram_tensor("wf", [2, 64, 128], F16, kind="ExternalInput")
    bf = nc.dram_tensor("bf", [2, 128, 1], F32, kind="ExternalInput")
    onesrow = nc.dram_tensor("onesrow", [1, 128], F16, kind="ExternalInput")
    yt = nc.dram_tensor("yt", [BS * 2, 128, N], F32, kind="ExternalOutput")

    with tile.TileContext(nc) as tc, contextlib.ExitStack() as ctx:
        pools = {
            "consts": ctx.enter_context(tc.tile_pool(name="consts", bufs=1)),
            "xt": ctx.enter_context(tc.tile_pool(name="xt", bufs=4)),
            "persist": ctx.enter_context(tc.tile_pool(name="persist", bufs=1)),
            "mt": ctx.enter_context(tc.tile_pool(name="mt", bufs=3)),
            "e": ctx.enter_context(tc.tile_pool(name="e", bufs=12)),
            "ar": ctx.enter_context(tc.tile_pool(name="ar", bufs=5)),
            "an": ctx.enter_context(tc.tile_pool(name="an", bufs=3)),
            "ot": ctx.enter_context(tc.tile_pool(name="ot", bufs=3)),
            "sc": ctx.enter_context(tc.tile_pool(name="sc", bufs=2, space="PSUM")),
            "av": ctx.enter_context(tc.tile_pool(name="av", bufs=2, space="PSUM")),
        }
        cp = pools["consts"]
        W = {
            "xt_ap": xt.ap(),
            "mt_ap": mt.ap(),
            "yt_ap": yt.ap(),
            "wq": [cp.tile([128, 128], F16, tag=f"wq{g}", name=f"wq{g}") for g in range(2)],
            "wk": [cp.tile([128, 128], F16, tag=f"wk{g}", name=f"wk{g}") for g in range(2)],
            "wv": [cp.tile([128, 128], F16, tag=f"wv{g}", name=f"wv{g}") for g in range(2)],
            "bq": [cp.tile([128, 1], F32, tag=f"bq{g}", name=f"bq{g}") for g in range(2)],
            "bk": [cp.tile([128, 1], F32, tag=f"bk{g}", name=f"bk{g}") for g in range(2)],
            "bvr4": [cp.tile([1, 512], F16, tag=f"bvr4{g}", name=f"bvr4{g}") for g in range(2)],
            "zrow16": cp.tile([1, 128], F16, tag="zrow16", name="zrow16"),
            "wo": [
                [cp.tile([64, 32], F16, tag=f"wo{p}{i}", name=f"wo{p}{i}") for i in range(2)]
                for p in range(4)
            ],
            "bo": [cp.tile([32, 1], F32, tag=f"bo{p}", name=f"bo{p}") for p in range(4)],
            "wf": [cp.tile([64, 128], F16, tag=f"wf{g}", name=f"wf{g}") for g in range(2)],
            "bf": [cp.tile([128, 1], F32, tag=f"bf{g}", name=f"bf{g}") for g in range(2)],
            "onesrow": cp.tile([1, 128], F16, tag="onesrow", name="onesrow"),
        }
        for g in range(2):
            nc.sync.dma_start(out=W["wq"][g][:], in_=wqk.ap()[g, 0])
            nc.sync.dma_start(out=W["wk"][g][:], in_=wqk.ap()[g, 1])
            nc.sync.dma_start(out=W["wv"][g][:], in_=wv.ap()[g])
            nc.sync.dma_start(out=W["bq"][g][:], in_=bqk.ap()[g, 0])
            nc.sync.dma_start(out=W["bk"][g][:], in_=bqk.ap()[g, 1])
            nc.sync.dma_start(out=W["bvr4"][g][:], in_=bvr4.ap()[g])
            nc.sync.dma_start(out=W["wf"][g][:], in_=wf.ap()[g])
            nc.sync.dma_start(out=W["bf"][g][:], in_=bf.ap()[g])
        for p in range(4):
            for i in range(2):
                nc.sync.dma_start(out=W["wo"][p][i][:], in_=wo.ap()[p, i])
            nc.sync.dma_start(out=W["bo"][p][:], in_=bo.ap()[p])
        nc.sync.dma_start(out=W["onesrow"][:], in_=onesrow.ap())
        nc.vector.memset(W["zrow16"][:], 0.0)

        pp = pools["persist"]
        for it in range(n_iters):
            sfx = ""
            qt = [
                [pp.tile([128, N], F16, tag=f"qt{b}{g}{sfx}", name=f"qt{b}{g}{sfx}") for g in range(2)]
                for b in range(BS)
            ]
            kt = [
                [pp.tile([128, N], F16, tag=f"kt{b}{g}{sfx}", name=f"kt{b}{g}{sfx}") for g in range(2)]
                for b in range(BS)
            ]
            vt = [
                [
                    pp.tile([128, 4, 2, 224], F16, tag=f"vt{b}{g}{sfx}", name=f"vt{b}{g}{sfx}")
                    for g in range(2)
                ]
                for b in range(BS)
            ]
            cc = [
                [pp.tile([64, N], F16, tag=f"cc{b}{g}{sfx}", name=f"cc{b}{g}{sfx}") for g in range(2)]
                for b in range(BS)
            ]
            if it == 0:
                for b in range(BS):
                    for g in range(2):
                        nc.vector.memset(vt[b][g][:, :, :, 32:64], 0.0)
                        nc.vector.memset(vt[b][g][:, :, :, 96:128], 0.0)
                        nc.vector.memset(vt[b][g][:, :, :, 160:192], 0.0)
                        nc.vector.memset(vt[b][g][:, :, :, 64:96], 1.0)
                        nc.vector.memset(vt[b][g][:, :, :, 192:224], 1.0)
            # staggered emission: QKV(b+1) groups interleave between pairs
            for g in range(2):
                _emit_qkv(nc, pools, W, 0, g, qt[0][g], kt[0][g], vt[0][g])
            for b in range(BS):
                for p in range(4):
                    if b + 1 < BS and p in (1, 3):
                        g = p // 2
                        _emit_qkv(
                            nc, pools, W, b + 1, g, qt[b + 1][g], kt[b + 1][g], vt[b + 1][g]
                        )
                    _emit_attn_p(nc, pools, W, b, p, qt[b], kt[b], vt[b], cc[b])
                for g in range(2):
                    _emit_out(nc, pools, W, b, g, cc[b])

    nc.compile()
    return nc


def prep_weights(inp):
    """Host-side packing of the (core-replicated) weight tensors."""
    f = np.asarray
    W = {}
    W["wqk"] = np.stack(
        [
            np.stack([f(inp["wq0"]), f(inp["wk0"])]),
            np.stack([f(inp["wq1"]), f(inp["wk1"])]),
        ]
    ).astype(np.float16)
    W["wv"] = np.stack([f(inp["wv0"]), f(inp["wv1"])]).astype(np.float16)
    W["bqk"] = np.stack(
        [
            np.stack([f(inp["bq0"]).reshape(128, 1), f(inp["bk0"]).reshape(128, 1)]),
            np.stack([f(inp["bq1"]).reshape(128, 1), f(inp["bk1"]).reshape(128, 1)]),
        ]
    ).astype(np.float32)
    W["bvr4"] = np.stack(
        [np.tile(f(inp["bv0"]), 4).reshape(1, 512), np.tile(f(inp["bv1"]), 4).reshape(1, 512)]
    ).astype(np.float16)
    W["wo"] = np.stack(
        [f(inp["wo00"]), f(inp["wo11"]), f(inp["wo01"]), f(inp["wo10"])]
    ).astype(np.float16).reshape(4, 2, 64, 32)
    W["bo"] = np.stack(
        [
            f(inp["bo00"]).reshape(32, 1),
            f(inp["bo11"]).reshape(32, 1),
            f(inp["bo01"]).reshape(32, 1),
            f(inp["bo10"]).reshape(32, 1),
        ]
    ).astype(np.float32)
    W["wf"] = np.stack([f(inp["wf0"]), f(inp["wf1"])]).astype(np.float16)
    W["bf"] = np.stack(
        [f(inp["bf0"]).reshape(128, 1), f(inp["bf1"]).reshape(128, 1)]
    ).astype(np.float32)
    W["onesrow"] = np.ones((1, 128), np.float16)
    return W


def prep_core_inputs(inp, W):
    """Build the 8 per-core in_maps (shards batch over cores)."""
    x = [np.asarray(inp["x0"], np.float32), np.asarray(inp["x1"], np.float32)]
    masks = [
        np.asarray(inp["m00"]),
        np.asarray(inp["m11"]),
        np.asarray(inp["m01"]),
        np.asarray(inp["m10"]),
    ]
    in_maps = []
    for ci in range(NCORES):
        xt = np.empty((BS * 2, 128, N), np.float16)
        mt = np.empty((BS * 4, 128, 4 * 1024), np.float16)
        for b in range(BS):
            gb = ci * BS + b
            for g in range(2):
                xt[b * 2 + g] = x[g][gb].T
            for p in range(4):
                mT = masks[p][gb].T.astype(np.float16)  # [k, q]
                ch = mT.reshape(4, 128, N)  # chunk c = k rows 128c..
                dup = np.stack([ch, ch], axis=1)  # [4, 2, 128, N]
                mt[b * 4 + p] = dup.transpose(2, 0, 1, 3).reshape(128, 4 * 1024)
        m = {"xt": xt, "mt": mt}
        m.update(W)
        in_maps.append(m)
    return in_maps


def postprocess(results):
    """Gather per-core yt [8,128,512] -> (out0, out1) full arrays."""
    out0 = np.empty((B, N, OUT_DIM), np.float32)
    out1 = np.empty((B, N, OUT_DIM), np.float32)
    for ci in range(NCORES):
        yt = results[ci]["yt"]
        for b in range(BS):
            gb = ci * BS + b
            out0[gb] = yt[b * 2 + 0].T
            out1[gb] = yt[b * 2 + 1].T
    return out0, out1


_NC_CACHE = {}


def get_nc(n_iters: int = 1):
    if n_iters not in _NC_CACHE:
        _NC_CACHE[n_iters] = build_nc(n_iters)
    return _NC_CACHE[n_iters]


def kernel(**inputs):
    from concourse import bass_utils

    nc = get_nc(1)
    W = prep_weights(inputs)
    in_maps = prep_core_inputs(inputs, W)
    res = bass_utils.run_bass_kernel_spmd(
        nc, in_maps, core_ids=list(range(NCORES)), trace=False
    )
    return postprocess(res.results)

